# revision 5
# baseline (speedup 1.0000x reference)
"""Embedding lookup + lc-connector MLP scatter kernel for 8 trn2 cores.

Strategy: data-parallel over the 16384 flattened (b, s) positions, 2048
per core. The wte table is replicated, augmented with one zero row at
index 32000. Host precomputes, per position, a single gather index:
the clipped token id, or 32000 (zero row) when the position is
overwritten by an lc update (last occurrence wins). The tiny MLP runs
feature-major on-device at position-aligned columns (dead columns are
exactly zero), its final matmul lands row-major in PSUM, and one
vector add merges it onto the gathered tile: gathered rows see +0,
overwritten rows are 0 + feature. All output writes are contiguous.
"""

import sys

for _p in ("/opt/trn_rl_repo", "/opt/pypackages"):
    if _p not in sys.path:
        sys.path.append(_p)

import numpy as np

import concourse.bass as bass
import concourse.bacc as bacc
import concourse.mybir as mybir
import concourse.tile as tile
from concourse import bass_utils

B, S = 4, 4096
VOCAB = 32000
H = 2048
ID = 64  # INT_DIM
NCORES = 8
NPOS = B * S           # 16384
PERCORE = NPOS // NCORES  # 2048
P = 128
NT = PERCORE // P      # 16 tiles per core
ZROW = VOCAB           # index of the zero row in the augmented table

_BUILD_CACHE = {}


def _build(n_wsb_bufs=4, n_ps_bufs=2, use_bias=True):
    key = (n_wsb_bufs, n_ps_bufs, use_bias)
    if key in _BUILD_CACHE:
        return _BUILD_CACHE[key]
    f32 = mybir.dt.float32
    nc = bacc.Bacc("TRN2", target_bir_lowering=False, debug=False,
                   enable_asserts=False, num_devices=NCORES)

    wte = nc.dram_tensor("wte", [VOCAB + 1, H], f32, kind="ExternalInput")
    gidx = nc.dram_tensor("gidx", [P, NT], mybir.dt.int32, kind="ExternalInput")
    xin = nc.dram_tensor("xin", [1, PERCORE], f32, kind="ExternalInput")
    ind = nc.dram_tensor("ind", [1, PERCORE], f32, kind="ExternalInput")
    w0 = nc.dram_tensor("w0", [1, ID], f32, kind="ExternalInput")
    w1 = nc.dram_tensor("w1", [ID, ID], f32, kind="ExternalInput")
    w2 = nc.dram_tensor("w2", [ID, ID], f32, kind="ExternalInput")
    wo = nc.dram_tensor("wo", [ID, H], f32, kind="ExternalInput")
    b0 = nc.dram_tensor("b0", [1, ID], f32, kind="ExternalInput")
    b1 = nc.dram_tensor("b1", [1, ID], f32, kind="ExternalInput")
    b2 = nc.dram_tensor("b2", [1, ID], f32, kind="ExternalInput")
    bo = nc.dram_tensor("bo", [1, H], f32, kind="ExternalInput")
    outs = [nc.dram_tensor(f"out{t}", [P, H], f32, kind="ExternalOutput")
            for t in range(NT)]

    GELU = mybir.ActivationFunctionType.Gelu
    CHUNK = 512
    NCH = PERCORE // CHUNK  # 4

    with tile.TileContext(nc) as tc:
        with (
            tc.tile_pool(name="const", bufs=1) as cp,
            tc.tile_pool(name="g", bufs=1) as gp,
            tc.tile_pool(name="wsb", bufs=n_wsb_bufs) as wp,
        ):
            # constants / small inputs
            idx_sb = cp.tile([P, NT], mybir.dt.int32)
            nc.sync.dma_start(out=idx_sb[:], in_=gidx[:])
            x_sb = cp.tile([1, PERCORE], f32)
            nc.sync.dma_start(out=x_sb[:], in_=xin[:])
            ind_sb = cp.tile([1, PERCORE], f32)
            nc.sync.dma_start(out=ind_sb[:], in_=ind[:])
            w0_sb = cp.tile([1, ID], f32)
            nc.sync.dma_start(out=w0_sb[:], in_=w0[:])
            w1_sb = cp.tile([ID, ID], f32)
            nc.sync.dma_start(out=w1_sb[:], in_=w1[:])
            w2_sb = cp.tile([ID, ID], f32)
            nc.sync.dma_start(out=w2_sb[:], in_=w2[:])
            wo_sb = cp.tile([ID, H], f32)
            nc.sync.dma_start(out=wo_sb[:], in_=wo[:])
            b0_sb = cp.tile([1, ID], f32)
            nc.sync.dma_start(out=b0_sb[:], in_=b0[:])
            b1_sb = cp.tile([1, ID], f32)
            nc.sync.dma_start(out=b1_sb[:], in_=b1[:])
            b2_sb = cp.tile([1, ID], f32)
            nc.sync.dma_start(out=b2_sb[:], in_=b2[:])
            bo_sb = cp.tile([1, H], f32)
            nc.sync.dma_start(out=bo_sb[:], in_=bo[:])

            # MLP interior, feature-major [64, n]
            g3 = gp.tile([ID, PERCORE], f32)
            with tc.tile_pool(name="mlp_ps", bufs=2, space="PSUM") as pa, \
                 tc.tile_pool(name="mlp_g", bufs=2) as gg:
                for k in range(NCH):
                    cs = slice(k * CHUNK, (k + 1) * CHUNK)
                    ps1 = pa.tile([ID, CHUNK], f32, tag="ps")
                    nc.tensor.matmul(ps1[:], w0_sb[:], x_sb[0:1, cs],
                                     start=True, stop=not use_bias)
                    if use_bias:
                        nc.tensor.matmul(ps1[:], b0_sb[:], ind_sb[0:1, cs],
                                         start=False, stop=True)
                    g1 = gg.tile([ID, CHUNK], f32, tag="g1")
                    nc.scalar.activation(g1[:], ps1[:], GELU)

                    ps2 = pa.tile([ID, CHUNK], f32, tag="ps")
                    nc.tensor.matmul(ps2[:], w1_sb[:], g1[:],
                                     start=True, stop=not use_bias)
                    if use_bias:
                        nc.tensor.matmul(ps2[:], b1_sb[:], ind_sb[0:1, cs],
                                         start=False, stop=True)
                    g2 = gg.tile([ID, CHUNK], f32, tag="g2")
                    nc.scalar.activation(g2[:], ps2[:], GELU)

                    ps3 = pa.tile([ID, CHUNK], f32, tag="ps")
                    nc.tensor.matmul(ps3[:], w2_sb[:], g2[:],
                                     start=True, stop=not use_bias)
                    if use_bias:
                        nc.tensor.matmul(ps3[:], b2_sb[:], ind_sb[0:1, cs],
                                         start=False, stop=True)
                    nc.scalar.activation(g3[:, cs], ps3[:], GELU)

            # per-tile: gather + final matmul + merge + store
            with tc.tile_pool(name="big_ps", bufs=n_ps_bufs, space="PSUM") as pb:
                for t in range(NT):
                    wsb = wp.tile([P, H], f32, tag="wsb")
                    nc.gpsimd.indirect_dma_start(
                        out=wsb[:], out_offset=None, in_=wte[:],
                        in_offset=bass.IndirectOffsetOnAxis(
                            ap=idx_sb[:, t:t + 1], axis=0))
                    psb = pb.tile([P, H], f32, tag="psb")
                    lh = g3[:, t * P:(t + 1) * P]
                    for q in range(H // CHUNK):
                        qs = slice(q * CHUNK, (q + 1) * CHUNK)
                        nc.tensor.matmul(psb[:, qs], lh, wo_sb[:, qs],
                                         start=True, stop=not use_bias)
                        if use_bias:
                            nc.tensor.matmul(psb[:, qs],
                                             ind_sb[0:1, t * P:(t + 1) * P],
                                             bo_sb[0:1, qs],
                                             start=False, stop=True)
                    for q in range(H // CHUNK):
                        qs = slice(q * CHUNK, (q + 1) * CHUNK)
                        nc.vector.tensor_add(wsb[:, qs], wsb[:, qs], psb[:, qs])
                    nc.sync.dma_start(out=outs[t][:], in_=wsb[:])

    nc.compile()
    _BUILD_CACHE[key] = nc
    return nc


def _prepare_in_maps(inputs):
    ids = np.clip(np.asarray(inputs["input_ids"]).astype(np.int64),
                  0, VOCAB - 1).reshape(-1).astype(np.int32)
    pb = np.asarray(inputs["pos_b"]).astype(np.int64)
    ps_ = np.asarray(inputs["pos_s"]).astype(np.int64)
    lcv = np.asarray(inputs["lc_values"], dtype=np.float32).reshape(-1)

    flat = pb * S + ps_
    order = np.argsort(flat, kind="stable")
    sf = flat[order]
    is_last = np.ones(len(sf), dtype=bool)
    if len(sf) > 1:
        is_last[:-1] = sf[:-1] != sf[1:]
    win_pos = sf[is_last]
    win_j = order[is_last]

    gather_ids = ids.copy()
    gather_ids[win_pos] = ZROW
    xfull = np.zeros(NPOS, np.float32)
    xfull[win_pos] = lcv[win_j]
    indfull = np.zeros(NPOS, np.float32)
    indfull[win_pos] = 1.0

    wte_aug = np.concatenate(
        [np.asarray(inputs["wte"], dtype=np.float32),
         np.zeros((1, H), np.float32)], axis=0)
    w = {k: np.ascontiguousarray(np.asarray(inputs[k], dtype=np.float32))
         for k in ("W0", "W1", "W2", "Wout")}
    bz = {k: np.asarray(inputs[k], dtype=np.float32).reshape(1, -1)
          for k in ("b0", "b1", "b2", "bout")}

    in_maps = []
    for c in range(NCORES):
        lo = c * PERCORE
        sl = slice(lo, lo + PERCORE)
        in_maps.append({
            "wte": wte_aug,
            "gidx": np.ascontiguousarray(
                gather_ids[sl].reshape(NT, P).T),
            "xin": xfull[sl].reshape(1, PERCORE),
            "ind": indfull[sl].reshape(1, PERCORE),
            "w0": w["W0"].reshape(1, ID), "w1": w["W1"], "w2": w["W2"],
            "wo": w["Wout"], "b0": bz["b0"], "b1": bz["b1"],
            "b2": bz["b2"], "bo": bz["bout"],
        })
    return in_maps


def run(inputs, trace=False, **kw):
    use_bias = any(
        np.abs(np.asarray(inputs[k], dtype=np.float32)).max() > 0
        for k in ("b0", "b1", "b2", "bout"))
    nc = _build(use_bias=use_bias)
    in_maps = _prepare_in_maps(inputs)
    res = bass_utils.run_bass_kernel_spmd(
        nc, in_maps, core_ids=list(range(NCORES)), trace=trace, **kw)
    out = np.empty((NPOS, H), np.float32)
    for c in range(NCORES):
        for t in range(NT):
            r0 = c * PERCORE + t * P
            out[r0:r0 + P] = res.results[c][f"out{t}"]
    return out.reshape(B, S, H), res


def kernel(**inputs):
    out, _ = run(inputs)
    return out


# revision 7
# speedup vs baseline: 1.1483x; 1.1483x over previous
"""Embedding lookup + lc-connector MLP scatter kernel for 8 trn2 cores.

Strategy: data-parallel over the 16384 flattened (b, s) positions, 2048
per core. The wte table is replicated, augmented with one zero row at
index 32000. Host precomputes, per position, a single gather index:
the clipped token id, or 32000 (zero row) when the position is
overwritten by an lc update (last occurrence wins). The tiny MLP runs
feature-major on-device at position-aligned columns (dead columns are
exactly zero), its final matmul lands row-major in PSUM, and one
vector add merges it onto the gathered tile: gathered rows see +0,
overwritten rows are 0 + feature. All output writes are contiguous.
"""

import sys

for _p in ("/opt/trn_rl_repo", "/opt/pypackages"):
    if _p not in sys.path:
        sys.path.append(_p)

import numpy as np

import concourse.bass as bass
import concourse.bacc as bacc
import concourse.mybir as mybir
import concourse.tile as tile
from concourse import bass_utils

B, S = 4, 4096
VOCAB = 32000
H = 2048
ID = 64  # INT_DIM
NCORES = 8
NPOS = B * S           # 16384
PERCORE = NPOS // NCORES  # 2048
P = 128
NT = PERCORE // P      # 16 tiles per core
ZROW = VOCAB           # index of the zero row in the augmented table

_BUILD_CACHE = {}


def _build(n_wsb_bufs=4, n_ps_bufs=2, use_bias=True):
    key = (n_wsb_bufs, n_ps_bufs, use_bias)
    if key in _BUILD_CACHE:
        return _BUILD_CACHE[key]
    f32 = mybir.dt.float32
    nc = bacc.Bacc("TRN2", target_bir_lowering=False, debug=False,
                   enable_asserts=False, num_devices=NCORES)

    wte = nc.dram_tensor("wte", [VOCAB + 1, H], f32, kind="ExternalInput")
    gidx = nc.dram_tensor("gidx", [P, NT], mybir.dt.int32, kind="ExternalInput")
    xin = nc.dram_tensor("xin", [1, PERCORE], f32, kind="ExternalInput")
    ind = nc.dram_tensor("ind", [1, PERCORE], f32, kind="ExternalInput")
    w0 = nc.dram_tensor("w0", [1, ID], f32, kind="ExternalInput")
    w1 = nc.dram_tensor("w1", [ID, ID], f32, kind="ExternalInput")
    w2 = nc.dram_tensor("w2", [ID, ID], f32, kind="ExternalInput")
    wo = nc.dram_tensor("wo", [ID, H], f32, kind="ExternalInput")
    b0 = nc.dram_tensor("b0", [1, ID], f32, kind="ExternalInput")
    b1 = nc.dram_tensor("b1", [1, ID], f32, kind="ExternalInput")
    b2 = nc.dram_tensor("b2", [1, ID], f32, kind="ExternalInput")
    bo = nc.dram_tensor("bo", [1, H], f32, kind="ExternalInput")
    outs = [nc.dram_tensor(f"out{t}", [P, H], f32, kind="ExternalOutput")
            for t in range(NT)]

    GELU = mybir.ActivationFunctionType.Gelu
    CHUNK = 512
    NCH = PERCORE // CHUNK  # 4

    with tile.TileContext(nc) as tc:
        with (
            tc.tile_pool(name="const", bufs=1) as cp,
            tc.tile_pool(name="g", bufs=1) as gp,
            tc.tile_pool(name="wsb", bufs=n_wsb_bufs) as wp,
        ):
            # constants / small inputs
            idx_sb = cp.tile([P, NT], mybir.dt.int32)
            nc.sync.dma_start(out=idx_sb[:], in_=gidx[:])
            x_sb = cp.tile([1, PERCORE], f32)
            nc.sync.dma_start(out=x_sb[:], in_=xin[:])
            ind_sb = cp.tile([1, PERCORE], f32)
            nc.sync.dma_start(out=ind_sb[:], in_=ind[:])
            w0_sb = cp.tile([1, ID], f32)
            nc.sync.dma_start(out=w0_sb[:], in_=w0[:])
            w1_sb = cp.tile([ID, ID], f32)
            nc.sync.dma_start(out=w1_sb[:], in_=w1[:])
            w2_sb = cp.tile([ID, ID], f32)
            nc.sync.dma_start(out=w2_sb[:], in_=w2[:])
            # bf16 final matmul: one rounding of (g3, Wout); PE runs 1
            # pass instead of fp32's 2. Falls back to fp32 when biases
            # are active (mixed-dtype operands would complicate).
            bf16_final = not use_bias
            wo_dt = mybir.dt.bfloat16 if bf16_final else f32
            wo_sb = cp.tile([ID, H], wo_dt)
            if bf16_final:
                nc.gpsimd.dma_start(out=wo_sb[:], in_=wo[:])  # SWDGE casts
            else:
                nc.sync.dma_start(out=wo_sb[:], in_=wo[:])
            b0_sb = cp.tile([1, ID], f32)
            nc.sync.dma_start(out=b0_sb[:], in_=b0[:])
            b1_sb = cp.tile([1, ID], f32)
            nc.sync.dma_start(out=b1_sb[:], in_=b1[:])
            b2_sb = cp.tile([1, ID], f32)
            nc.sync.dma_start(out=b2_sb[:], in_=b2[:])
            bo_sb = cp.tile([1, H], f32)
            nc.sync.dma_start(out=bo_sb[:], in_=bo[:])

            # MLP interior, feature-major [64, n]
            g3 = gp.tile([ID, PERCORE], wo_dt)
            with tc.tile_pool(name="mlp_ps", bufs=2, space="PSUM") as pa, \
                 tc.tile_pool(name="mlp_g", bufs=2) as gg:
                for k in range(NCH):
                    cs = slice(k * CHUNK, (k + 1) * CHUNK)
                    ps1 = pa.tile([ID, CHUNK], f32, tag="ps")
                    nc.tensor.matmul(ps1[:], w0_sb[:], x_sb[0:1, cs],
                                     start=True, stop=not use_bias)
                    if use_bias:
                        nc.tensor.matmul(ps1[:], b0_sb[:], ind_sb[0:1, cs],
                                         start=False, stop=True)
                    g1 = gg.tile([ID, CHUNK], f32, tag="g1")
                    nc.scalar.activation(g1[:], ps1[:], GELU)

                    ps2 = pa.tile([ID, CHUNK], f32, tag="ps")
                    nc.tensor.matmul(ps2[:], w1_sb[:], g1[:],
                                     start=True, stop=not use_bias)
                    if use_bias:
                        nc.tensor.matmul(ps2[:], b1_sb[:], ind_sb[0:1, cs],
                                         start=False, stop=True)
                    g2 = gg.tile([ID, CHUNK], f32, tag="g2")
                    nc.scalar.activation(g2[:], ps2[:], GELU)

                    ps3 = pa.tile([ID, CHUNK], f32, tag="ps")
                    nc.tensor.matmul(ps3[:], w2_sb[:], g2[:],
                                     start=True, stop=not use_bias)
                    if use_bias:
                        nc.tensor.matmul(ps3[:], b2_sb[:], ind_sb[0:1, cs],
                                         start=False, stop=True)
                    nc.scalar.activation(g3[:, cs], ps3[:], GELU)

            # per-tile: gather + final matmul + merge + store
            with tc.tile_pool(name="big_ps", bufs=n_ps_bufs, space="PSUM") as pb:
                for t in range(NT):
                    wsb = wp.tile([P, H], f32, tag="wsb")
                    nc.gpsimd.indirect_dma_start(
                        out=wsb[:], out_offset=None, in_=wte[:],
                        in_offset=bass.IndirectOffsetOnAxis(
                            ap=idx_sb[:, t:t + 1], axis=0))
                    psb = pb.tile([P, H], f32, tag="psb")
                    lh = g3[:, t * P:(t + 1) * P]
                    for q in range(H // CHUNK):
                        qs = slice(q * CHUNK, (q + 1) * CHUNK)
                        nc.tensor.matmul(psb[:, qs], lh, wo_sb[:, qs],
                                         start=True, stop=not use_bias)
                        if use_bias:
                            nc.tensor.matmul(psb[:, qs],
                                             ind_sb[0:1, t * P:(t + 1) * P],
                                             bo_sb[0:1, qs],
                                             start=False, stop=True)
                    for q in range(H // CHUNK):
                        qs = slice(q * CHUNK, (q + 1) * CHUNK)
                        nc.vector.tensor_add(wsb[:, qs], wsb[:, qs], psb[:, qs])
                    nc.sync.dma_start(out=outs[t][:], in_=wsb[:])

    nc.compile()
    _BUILD_CACHE[key] = nc
    return nc


def _prepare_in_maps(inputs):
    ids = np.clip(np.asarray(inputs["input_ids"]).astype(np.int64),
                  0, VOCAB - 1).reshape(-1).astype(np.int32)
    pb = np.asarray(inputs["pos_b"]).astype(np.int64)
    ps_ = np.asarray(inputs["pos_s"]).astype(np.int64)
    lcv = np.asarray(inputs["lc_values"], dtype=np.float32).reshape(-1)

    flat = pb * S + ps_
    order = np.argsort(flat, kind="stable")
    sf = flat[order]
    is_last = np.ones(len(sf), dtype=bool)
    if len(sf) > 1:
        is_last[:-1] = sf[:-1] != sf[1:]
    win_pos = sf[is_last]
    win_j = order[is_last]

    gather_ids = ids.copy()
    gather_ids[win_pos] = ZROW
    xfull = np.zeros(NPOS, np.float32)
    xfull[win_pos] = lcv[win_j]
    indfull = np.zeros(NPOS, np.float32)
    indfull[win_pos] = 1.0

    wte_aug = np.concatenate(
        [np.asarray(inputs["wte"], dtype=np.float32),
         np.zeros((1, H), np.float32)], axis=0)
    w = {k: np.ascontiguousarray(np.asarray(inputs[k], dtype=np.float32))
         for k in ("W0", "W1", "W2", "Wout")}
    bz = {k: np.asarray(inputs[k], dtype=np.float32).reshape(1, -1)
          for k in ("b0", "b1", "b2", "bout")}

    in_maps = []
    for c in range(NCORES):
        lo = c * PERCORE
        sl = slice(lo, lo + PERCORE)
        in_maps.append({
            "wte": wte_aug,
            "gidx": np.ascontiguousarray(
                gather_ids[sl].reshape(NT, P).T),
            "xin": xfull[sl].reshape(1, PERCORE),
            "ind": indfull[sl].reshape(1, PERCORE),
            "w0": w["W0"].reshape(1, ID), "w1": w["W1"], "w2": w["W2"],
            "wo": w["Wout"], "b0": bz["b0"], "b1": bz["b1"],
            "b2": bz["b2"], "bo": bz["bout"],
        })
    return in_maps


def run(inputs, trace=False, **kw):
    use_bias = any(
        np.abs(np.asarray(inputs[k], dtype=np.float32)).max() > 0
        for k in ("b0", "b1", "b2", "bout"))
    nc = _build(use_bias=use_bias)
    in_maps = _prepare_in_maps(inputs)
    res = bass_utils.run_bass_kernel_spmd(
        nc, in_maps, core_ids=list(range(NCORES)), trace=trace, **kw)
    out = np.empty((NPOS, H), np.float32)
    for c in range(NCORES):
        for t in range(NT):
            r0 = c * PERCORE + t * P
            out[r0:r0 + P] = res.results[c][f"out{t}"]
    return out.reshape(B, S, H), res


def kernel(**inputs):
    out, _ = run(inputs)
    return out
